# revision 35
# baseline (speedup 1.0000x reference)
"""Trainium2 Bass kernel for local windowed per-channel attention (sparse_attention).

Reference computation (per batch b, channel c, position (h,w)):
    q = W_q x ; k = W_k x_pad ; v = W_v x_pad           (1x1 convs)
    s[i,j]  = q[h,w] * (k[h+i, w+j] + bias[c, i or j])  over a 7x7 window
    out     = sum_ij softmax_ij(s) * v[h+i, w+j]

Sharding: spatial, 8 ways — core = (batch, 12-row slab). Fully independent
per core (no collectives). Host pre-pads each slab with the 3-row/col halo.

Per-core dataflow (channels on partitions, 2 channel-tiles of 128):
  TensorE : q/k/v GEMMs on fp16 inputs; den|num 49-tap reduction via
            identity-matmul accumulation into one 3-bank PSUM region
            [den(576) | num(576)] per tile.  The region is pre-zeroed by
            start=True matmuls of a zero tile (a start=True matmul marks
            its whole 2KB PSUM bank pending-zero, so the den-tail and
            num-head streams that share the middle bank must both
            accumulate with start=False onto pre-zeroed banks).
  ScalarE : exp over contiguous e-chunks (strided activation APs cost +29%
            on hw), plus PSUM evictions of the k/v maps.
  VectorE : score mults q*kb and most weight mults e*v (fp16/bf16 DVE 2x
            via shifted-window APs), biased-k stacks (4x tensor_scalar),
            q eviction, reciprocal.
  GpSimd  : tuned slice of the mults + the final divide.
  DMA     : kb1/v1 one-column-shifted copies (4B alignment for odd taps),
            batched one DMA per stack.

The repeat loop used by the benchmark emits bodies with parity-alternated
k/v/q buffers inside an unrolled For_i, so body n+1's head (input DMA,
GEMMs, stack production — all DVE-light) overlaps body n's attention
phase.  em tiles ride one shared ring across bodies.  Tile-0 kb slabs
hold only the 12-row window group g reads (rows g..g+11); tile-1 slabs
need all 18 rows (row-shifted taps) and stay full.

em layout per (group, tile): [e0..e6 (7x576 fp16 scores, exp'd in place
to bf16) | m0..m6 (bf16)], with mt0 slot order [j=0,2,4,6, j=1,3,5] so
each parity half is a contiguous exp chunk.
"""
import os
import numpy as np
from contextlib import ExitStack

from concourse import bass, bacc, mybir, tile
from concourse.bass_utils import run_bass_kernel_spmd

F32 = mybir.dt.float32
F16 = mybir.dt.float16
BF16 = mybir.dt.bfloat16

K, PAD = 7, 3
B, CIN, COUT, H, W = 2, 256, 256, 48, 48
ROWS = 12                 # output rows per core
SH, SW = ROWS + 2 * PAD, W + 2 * PAD   # 18, 54 padded slab
NPOS = ROWS * W           # 576 output positions per core
NPAD = SH * SW            # 972 padded positions
NQ = ROWS * SW            # 648 q-map positions
N_CORES = 8
EBLK = K * NPOS           # 4032: e-block (and m-block) width per group
NSLB = ROWS * SW + 8      # 656: windowed tile-0 kb slab (12 rows + margin)
NWIN = ROWS * SW          # 648 written cols per windowed slab

F32R = mybir.dt.float32r
GEMM_DT = os.environ.get("GEMM_DT", "f16")

# ---- engine assignment knobs (tuned on hw) ----
POOL_M0 = set(int(c) for c in os.environ.get("POOL_M0", ""))
POOL_S1 = set(int(c) for c in os.environ.get("POOL_S1", ""))
POOL_M1 = set()
for tok in os.environ.get("POOL_M1", "").split(","):
    if tok:
        POOL_M1.add((int(tok[0]), int(tok[1])))
POOL_DIV = bool(int(os.environ.get("POOL_DIV", "0")))  # GpSimd can't read PSUM
ACT_QEV = bool(int(os.environ.get("ACT_QEV", "1")))
E_BUFS = int(os.environ.get("E_BUFS", "7"))
M_BUFS = int(os.environ.get("M_BUFS", "3"))
# num-accum emitted NUM_DELAY groups behind its m-mult, so a slow (Pool)
# m-mult never stalls PE's in-order stream; needs M_BUFS >= 2 + NUM_DELAY
NUM_DELAY = int(os.environ.get("NUM_DELAY", "1"))
# delay the m-mults one group behind the scores, so DVE runs
# score(g+1) during exp(g) instead of stalling for it
M_DELAY = int(os.environ.get("M_DELAY", "1"))
MT_ILV = bool(int(os.environ.get("MT_ILV", "0")))
WARM_MM = int(os.environ.get("WARM_MM", "16"))
PAIR = bool(int(os.environ.get("PAIR", "1")))
UNROLL = int(os.environ.get("UNROLL", "8"))
STAGGER = bool(int(os.environ.get("STAGGER", "0")))
SPLIT_MT1 = bool(int(os.environ.get("SPLIT_MT1", "0")))
EXP1 = bool(int(os.environ.get("EXP1", "0")))  # one exp instr per group
POOL_STACK = bool(int(os.environ.get("POOL_STACK", "0")))
DIV_VIA = os.environ.get("DIV_VIA", "dve")  # dve | pool
SKIP = set(os.environ.get("SKIP", "").split(",")) - {""}

JEVEN = [0, 2, 4, 6]
JODD = [1, 3, 5]

_CACHED = {}


def _fap(t, offset, dims):
    """Custom free-dim AP on a tile: dims = [[stride, size], ...]."""
    a = t[:]
    return bass.AP(a.tensor, a.offset + offset, [list(a.ap[0])] + dims)


class _Ctx:
    pass


def _emit_prologue(nc, tc, stk):
    cx = _Ctx()
    cx.const = stk.enter_context(tc.tile_pool(name="const", bufs=1))
    cx.work = stk.enter_context(tc.tile_pool(name="work", bufs=1))
    cx.gpsum = stk.enter_context(
        tc.tile_pool(name="gpsum", bufs=1, space="PSUM"))
    cx.apsum = stk.enter_context(
        tc.tile_pool(name="apsum", bufs=1, space="PSUM"))
    cx.ring = stk.enter_context(tc.tile_pool(name="ring", bufs=2))

    GDT = {"f16": F16, "f32r": F32R, "f32": F32}[GEMM_DT]
    cx.gdt = GDT
    # beta(2K) and identity(128) share one f32 dram tensor / one DMA
    cx.bi = cx.const.tile([128, 2 * K + 128], F32, name="bi")
    cx.idb = cx.const.tile([128, 128], BF16, name="idb")
    cx.zz = cx.const.tile([128, 512], BF16, name="zz")    # PSUM pre-zero src
    cx.x_sb = [cx.const.tile([128, NPAD], GDT, name=f"x_sb{kt}")
               for kt in range(2)]
    # w layout: [kt][k|q|v][mt] blocks of 128 cols -> [128, 2*3*2*128]
    cx.w_sb = cx.const.tile([128, 12 * 128], GDT, name="w_sb")
    cx.warm = cx.const.tile([128, 2], F32, name="warm")
    cx.k0 = [cx.work.tile([128, NPAD], F16, name=f"k0_{mt}")
             for mt in range(2)]
    cx.kb0 = {}
    cx.kb1 = {}
    cx.v0 = {}
    cx.v1 = {}
    cx.q_sb = {}
    for p in range(2 if PAIR else 1):
        cx.kb0[p, 0] = cx.work.tile([128, K * NSLB], F16, name=f"kb0_{p}0")
        cx.kb0[p, 1] = cx.work.tile([128, K * NPAD], F16, name=f"kb0_{p}1")
        cx.kb1[p, 0] = cx.work.tile([128, K * NSLB], F16, name=f"kb1_{p}0")
        cx.kb1[p, 1] = cx.work.tile([128, 3 * NPAD], F16, name=f"kb1_{p}1")
        for mt in range(2):
            cx.v0[p, mt] = cx.work.tile([128, NPAD], F16, name=f"v0_{p}{mt}")
            cx.v1[p, mt] = cx.work.tile([128, NPAD], F16, name=f"v1_{p}{mt}")
            cx.q_sb[p, mt] = cx.work.tile([128, NPOS], F16, name=f"q_{p}{mt}")
    # dn = [den(576) | num(576)] f32 = 3 PSUM banks per channel tile
    cx.dn = [cx.apsum.tile([128, 2 * NPOS], F32, name=f"dn{mt}")
             for mt in range(2)]
    return cx


def _emit_body(nc, tc, cx, dram, par_i, warm=False):
    x_d, w_d, bi_d, out_d = dram
    MULT = mybir.AluOpType.mult
    p = par_i if PAIR else 0
    cx.body_n = getattr(cx, "body_n", -1) + 1
    bn = cx.body_n
    beta = cx.bi       # beta columns [0 : 2K]
    kb0 = [cx.kb0[p, mt] for mt in range(2)]
    kb1 = [cx.kb1[p, mt] for mt in range(2)]
    v0 = [cx.v0[p, mt] for mt in range(2)]
    v1 = [cx.v1[p, mt] for mt in range(2)]
    q_sb = [cx.q_sb[p, mt] for mt in range(2)]

    # ---- input DMAs (batched: 4 loads) ----
    if "dma" not in SKIP:
        nc.sync.dma_start(cx.bi[:], bi_d[:, :])
        for kt in range(2):
            nc.sync.dma_start(cx.x_sb[kt][:],
                              x_d[kt * 128:(kt + 1) * 128, :])
        nc.sync.dma_start(cx.w_sb[:], w_d[:, :])
    if warm:
        nc.vector.tensor_copy(cx.idb[:], cx.bi[:, 2 * K:])
        nc.vector.memset(cx.zz[:], 0.0)
        nc.scalar.activation(cx.warm[:], cx.bi[:, :2],
                             mybir.ActivationFunctionType.Exp)
        if WARM_MM:
            for wi in range(WARM_MM):
                nc.tensor.matmul(cx.dn[0][:, :128], cx.idb[:], cx.idb[:],
                                 start=(wi == 0), stop=(wi == WARM_MM - 1))

    def wsl(nm, kt, mt):
        base = ((kt * 3) + {"k": 0, "q": 1, "v": 2}[nm]) * 2 + mt
        return cx.w_sb[:, base * 128:(base + 1) * 128]

    # ---- GEMMs + map production ----
    for mt in range(2) if "gemm" not in SKIP else ():
        # k map
        kp = cx.gpsum.tile([128, NPAD], F32, tag="gp", bufs=1, name=f"kp{mt}")
        for kt in range(2):
            for c0, c1 in ((0, 512), (512, NPAD)):
                nc.tensor.matmul(kp[:, c0:c1], wsl("k", kt, mt),
                                 cx.x_sb[kt][:, c0:c1],
                                 start=(kt == 0), stop=(kt == 1))
        nc.scalar.copy(cx.k0[mt][:], kp[:])
        # biased stacks: fp16 tensor_scalar adds run the 4x DVE mode
        if "stack" not in SKIP:
            seng = nc.gpsimd if POOL_STACK else nc.vector
            for t in range(K):
                if mt == 0:
                    seng.tensor_scalar_add(
                        kb0[0][:, t * NSLB:t * NSLB + NWIN],
                        cx.k0[0][:, t * SW:t * SW + NWIN],
                        beta[:, t:t + 1])
                else:
                    seng.tensor_scalar_add(
                        kb0[1][:, t * NPAD:(t + 1) * NPAD], cx.k0[1][:],
                        beta[:, K + t:K + t + 1])
            # 1-col-shifted copies, one batched DMA per stack
            if mt == 0:
                nc.sync.dma_start(
                    _fap(kb1[0], 0, [[NSLB, K], [1, NWIN - 1]]),
                    _fap(kb0[0], 1, [[NSLB, K], [1, NWIN - 1]]))
            else:
                nc.sync.dma_start(
                    _fap(kb1[1], 0, [[NPAD, 3], [1, NPAD - 2]]),
                    _fap(kb0[1], NPAD + 1, [[2 * NPAD, 3], [1, NPAD - 2]]))
        # q map: only the 12 center rows (cols incl. pad)
        qp = cx.gpsum.tile([128, NQ], F32, tag="gp", bufs=1, name=f"qp{mt}")
        for kt in range(2):
            for c0, c1 in ((0, 512), (512, NQ)):
                nc.tensor.matmul(qp[:, c0:c1], wsl("q", kt, mt),
                                 cx.x_sb[kt][:, PAD * SW + c0:PAD * SW + c1],
                                 start=(kt == 0), stop=(kt == 1))
        qsrc = _fap(qp, PAD, [[SW, ROWS], [1, W]])
        qdst = q_sb[mt][:].rearrange("p (h w) -> p h w", h=ROWS)
        if ACT_QEV:
            nc.scalar.copy(qdst, qsrc)
        else:
            nc.vector.tensor_copy(qdst, qsrc)
        # v map
        vp = cx.gpsum.tile([128, NPAD], F32, tag="gp", bufs=1, name=f"vp{mt}")
        for kt in range(2):
            for c0, c1 in ((0, 512), (512, NPAD)):
                nc.tensor.matmul(vp[:, c0:c1], wsl("v", kt, mt),
                                 cx.x_sb[kt][:, c0:c1],
                                 start=(kt == 0), stop=(kt == 1))
        nc.scalar.copy(v0[mt][:], vp[:])
        nc.sync.dma_start(v1[mt][:, :NPAD - 2], v0[mt][:, 1:NPAD - 1])

    # ---- attention ----
    # pre-zero dn via start=True matmuls of the zero tile (marks+clears the
    # 3 banks); all real accumulation runs start=False on top.
    for mt in range(2):
        for c0, c1 in ((0, 512), (512, 1024), (1024, 2 * NPOS)):
            nc.tensor.matmul(cx.dn[mt][:, c0:c1], cx.idb[:],
                             cx.zz[:, :c1 - c0], start=True, stop=True,
                             skip_group_check=True)
    order = ([(g, mt) for g in range(K) for mt in range(2)] if MT_ILV
             else [(g, mt) for mt in range(2) for g in range(K)])
    done = set()
    pend_num = []
    pend_m = []
    for g, mt in order:
        # separate e/m ring tiles: e_t frees after (m-mult, den-accum), m_t
        # after num-accum — independent release deepens the pipeline
        e_t = cx.ring.tile([128, EBLK], BF16, tag="e", bufs=E_BUFS,
                           name=f"e{mt}_{g}_{bn}")
        m_t = cx.ring.tile([128, EBLK], BF16, tag="m", bufs=M_BUFS,
                           name=f"m{mt}_{g}_{bn}")
        last = (g == K - 1)

        def score(half, e_t=e_t, mt=mt, g=g):
            if mt == 0:
                # groups = j (col shift, parity via kb1), slots = i via the
                # slab stride: one aligned instr covers all 7 i-taps
                par = g % 2
                if SPLIT_MT1:
                    i0, ni = ((0, 4), (4, 3))[half]
                elif half:
                    return
                else:
                    i0, ni = 0, K
                kb_ap = _fap(kb1[0] if par else kb0[0],
                             (g - par) + i0 * NSLB,
                             [[NSLB, ni], [SW, ROWS], [1, W]])
                q_ap = _fap(q_sb[0], 0, [[0, ni], [W, ROWS], [1, W]])
                s_ap = _fap(e_t, i0 * NPOS,
                            [[NPOS, ni], [W, ROWS], [1, W]]).bitcast(F16)
                nc.vector.tensor_tensor(s_ap, kb_ap, q_ap, MULT)
            else:
                par = g % 2
                if par:
                    kb_t = kb1[1]
                    bb = ((g - 1) // 2) * NPAD + (g - 1)
                else:
                    kb_t, bb = kb0[1], g * NPAD + g
                if SPLIT_MT1:
                    i0, ni = ((0, 4), (4, 3))[half]
                elif half:
                    return
                else:
                    i0, ni = 0, K
                kb_ap = _fap(kb_t, bb + i0 * SW,
                             [[SW, ni], [SW, ROWS], [1, W]])
                q_ap = _fap(q_sb[1], 0, [[0, ni], [W, ROWS], [1, W]])
                s_ap = _fap(e_t, i0 * NPOS,
                            [[NPOS, ni], [W, ROWS], [1, W]]).bitcast(F16)
                eng = nc.gpsimd if g in POOL_S1 else nc.vector
                eng.tensor_tensor(s_ap, kb_ap, q_ap, MULT)

        def expf(half, e_t=e_t):
            if EXP1:
                if half:
                    return
                s0, s1 = 0, EBLK
            else:
                s0, s1 = ((0, 4 * NPOS), (4 * NPOS, EBLK))[half]
            nc.scalar.activation(
                _fap(e_t, s0, [[1, s1 - s0]]),
                _fap(e_t, s0, [[1, s1 - s0]]).bitcast(F16),
                mybir.ActivationFunctionType.Exp)

        def mmul(half, e_t=e_t, m_t=m_t, mt=mt, g=g):
            if mt == 0:
                par = g % 2
                if SPLIT_MT1:
                    i0, ni = ((0, 4), (4, 3))[half]
                elif half:
                    return
                else:
                    i0, ni = 0, K
                e_ap = _fap(e_t, i0 * NPOS, [[NPOS, ni], [W, ROWS], [1, W]])
                v_ap = _fap(v1[0] if par else v0[0],
                            (g - par) + i0 * SW,
                            [[SW, ni], [SW, ROWS], [1, W]])
                m_ap = _fap(m_t, i0 * NPOS,
                            [[NPOS, ni], [W, ROWS], [1, W]])
                eng = (nc.gpsimd if g in POOL_M0 else nc.vector)
                eng.tensor_tensor(m_ap, e_ap, v_ap, MULT)
            else:
                par = g % 2
                if SPLIT_MT1:
                    i0, ni = ((0, 4), (4, 3))[half]
                elif half:
                    return
                else:
                    i0, ni = 0, K
                e_ap = _fap(e_t, i0 * NPOS, [[NPOS, ni], [W, ROWS], [1, W]])
                v_ap = _fap(v1[1] if par else v0[1],
                            g - par + i0 * SW,
                            [[SW, ni], [SW, ROWS], [1, W]])
                m_ap = _fap(m_t, i0 * NPOS,
                            [[NPOS, ni], [W, ROWS], [1, W]])
                eng = (nc.gpsimd if (g, half) in POOL_M1 else nc.vector)
                eng.tensor_tensor(m_ap, e_ap, v_ap, MULT)

        def accum(half, blk, e_t=e_t, m_t=m_t, mt=mt, last=last):
            # bind tiles at definition: the closure is queued in pend_num
            # and must keep THIS group's tiles, not the loop's latest.
            # chunks must stay within 512-col PSUM banks:
            # den at dn[0:576]   -> (0,512),(512,576)
            # num at dn[576:1152] -> (576,1024),(1024,1152)
            off = 0 if blk == 0 else NPOS
            chunks = ((0, 512), (512, NPOS)) if blk == 0 else \
                     ((0, 448), (448, NPOS))
            sls = (range(0, 4), range(4, K))[half]
            src_t = e_t if blk == 0 else m_t
            for sl in sls:
                for c0, c1 in chunks:
                    nc.tensor.matmul(
                        cx.dn[mt][:, off + c0:off + c1], cx.idb[:],
                        src_t[:, sl * NPOS + c0:sl * NPOS + c1],
                        start=False,
                        stop=(last and sl == K - 1 and half == 1),
                        skip_group_check=True)

        if "score" not in SKIP:
            score(0)
            score(1)
        if "exp" not in SKIP:
            expf(0)
            expf(1)
        if M_DELAY == 0:
            if "mmul" not in SKIP:
                mmul(0)
            if "accum" not in SKIP:
                accum(0, 0)
            if "mmul" not in SKIP:
                mmul(1)
            if "accum" not in SKIP:
                if NUM_DELAY == 0:
                    accum(0, EBLK)
                    accum(1, 0)
                    accum(1, EBLK)
                else:
                    accum(1, 0)
                    pend_num.append((accum, mt))
                    while len(pend_num) > NUM_DELAY:
                        fn, _ = pend_num.pop(0)
                        fn(0, EBLK)
                        fn(1, EBLK)
        else:
            if "accum" not in SKIP:
                accum(0, 0)
                accum(1, 0)
            pend_m.append((mmul, accum, mt))
            while len(pend_m) > M_DELAY:
                mf, af, _ = pend_m.pop(0)
                if "mmul" not in SKIP:
                    mf(0)
                if "accum" not in SKIP:
                    af(0, EBLK)
                if "mmul" not in SKIP:
                    mf(1)
                if "accum" not in SKIP:
                    af(1, EBLK)
        done.add((g, mt))
        if "out" in SKIP:
            continue
        if all((gg, mt) in done for gg in range(K)):
            while pend_m and pend_m[0][2] == mt:
                mf, af, _ = pend_m.pop(0)
                if "mmul" not in SKIP:
                    mf(0)
                if "accum" not in SKIP:
                    af(0, EBLK)
                if "mmul" not in SKIP:
                    mf(1)
                if "accum" not in SKIP:
                    af(1, EBLK)
            while pend_num and pend_num[0][1] == mt:
                fn, _ = pend_num.pop(0)
                fn(0, EBLK)
                fn(1, EBLK)
            rden = cx.ring.tile([128, NPOS], F32, tag="rden", bufs=2,
                                name=f"rden{mt}_{bn}")
            nc.vector.reciprocal(rden[:], cx.dn[mt][:, :NPOS])
            o_t = cx.ring.tile([128, NPOS], F32, tag="o", bufs=2,
                               name=f"o{mt}_{bn}")
            if DIV_VIA == "pool":
                # evict num to SBUF on ACT, divide on Pool (can't read PSUM)
                nev = cx.ring.tile([128, NPOS], F32, tag="nev", bufs=2,
                                   name=f"nev{mt}_{bn}")
                nc.scalar.copy(nev[:], cx.dn[mt][:, NPOS:])
                for c0, c1 in ((0, NPOS // 2), (NPOS // 2, NPOS)):
                    nc.gpsimd.tensor_tensor(o_t[:, c0:c1], nev[:, c0:c1],
                                            rden[:, c0:c1], MULT)
                    nc.sync.dma_start(out_d[mt * 128:(mt + 1) * 128, c0:c1],
                                      o_t[:, c0:c1])
            else:
                for c0, c1 in ((0, NPOS // 2), (NPOS // 2, NPOS)):
                    nc.vector.tensor_tensor(o_t[:, c0:c1],
                                            cx.dn[mt][:, NPOS + c0:NPOS + c1],
                                            rden[:, c0:c1], MULT)
                    nc.sync.dma_start(out_d[mt * 128:(mt + 1) * 128, c0:c1],
                                      o_t[:, c0:c1])


def _build_graph(repeat=1):
    nc = bacc.Bacc("TRN2", target_bir_lowering=False, debug=False,
                   num_devices=N_CORES)

    GDT = {"f16": F16, "f32r": F32R, "f32": F32}[GEMM_DT]
    dram = (
        nc.declare_dram_parameter("x_slab", [CIN, NPAD], GDT, isOutput=False),
        nc.declare_dram_parameter("w_pack", [128, 12 * 128], GDT,
                                  isOutput=False),
        nc.declare_dram_parameter("bi_pack", [128, 2 * K + 128], F32,
                                  isOutput=False),
        nc.declare_dram_parameter("out", [COUT, NPOS], F32, isOutput=True),
    )

    with tile.TileContext(nc) as tc:
        with ExitStack() as stk:
            cx = _emit_prologue(nc, tc, stk)
            _emit_body(nc, tc, cx, dram, 0, warm=True)
            left = repeat - 1
            if left > 0:
                u = min(UNROLL, left)
                trips, rem = divmod(left, u)
                if trips > 0:
                    with tc.For_i(0, trips, 1, staggered_reset=STAGGER):
                        for j in range(u):
                            _emit_body(nc, tc, cx, dram, (j + 1) % 2)
                for j in range(rem):
                    _emit_body(nc, tc, cx, dram, (j + 1) % 2)

    nc.compile()
    return nc


def _prep_host(x, w_q, w_k, w_v, rel_h, rel_w):
    gnp = {"f16": np.float16, "f32r": np.float32,
           "f32": np.float32}[GEMM_DT]
    x = np.ascontiguousarray(x, np.float32)
    beta = np.zeros((COUT, K), np.float32)
    beta[:COUT // 2] = rel_h.reshape(COUT // 2, K)
    beta[COUT // 2:] = rel_w.reshape(COUT // 2, K)
    bi_pack = np.empty((128, 2 * K + 128), np.float32)
    for mt in range(2):
        bi_pack[:, mt * K:(mt + 1) * K] = beta[mt * 128:(mt + 1) * 128]
    bi_pack[:, 2 * K:] = np.eye(128, dtype=np.float32)
    # w layout: [kt][k|q|v][mt] blocks of 128 cols
    w_pack = np.empty((128, 12 * 128), gnp)
    ws = {"k": w_k, "q": w_q, "v": w_v}
    for kt in range(2):
        for wi, nm in enumerate("kqv"):
            for mt in range(2):
                blk = ((kt * 3) + wi) * 2 + mt
                w_pack[:, blk * 128:(blk + 1) * 128] = \
                    ws[nm].T[kt * 128:(kt + 1) * 128,
                             mt * 128:(mt + 1) * 128].astype(gnp)
    common = {"w_pack": w_pack, "bi_pack": bi_pack}
    in_maps = []
    for core in range(N_CORES):
        b, r0 = divmod(core, 4)
        r0 *= ROWS
        slab = np.zeros((CIN, SH, SW), np.float32)
        lo, hi = r0 - PAD, r0 + ROWS + PAD
        clo, chi = max(lo, 0), min(hi, H)
        slab[:, clo - lo:chi - lo, PAD:PAD + W] = x[b, :, clo:chi, :]
        in_maps.append({"x_slab": slab.reshape(CIN, NPAD).astype(gnp),
                        **common})
    return in_maps


def kernel(x, w_q, w_k, w_v, rel_h, rel_w):
    if "nc" not in _CACHED:
        _CACHED["nc"] = _build_graph()
    nc = _CACHED["nc"]
    in_maps = _prep_host(x, w_q, w_k, w_v, rel_h, rel_w)
    res = run_bass_kernel_spmd(nc, in_maps, core_ids=list(range(N_CORES)))
    _CACHED["exec_time_ns"] = res.exec_time_ns
    out = np.empty((B, COUT, H, W), np.float32)
    for core in range(N_CORES):
        b, r0 = divmod(core, 4)
        r0 *= ROWS
        out[b, :, r0:r0 + ROWS, :] = \
            res.results[core]["out"].reshape(COUT, ROWS, W)
    return out


# revision 37
# speedup vs baseline: 1.2167x; 1.2167x over previous
"""Trainium2 Bass kernel for local windowed per-channel attention (sparse_attention).

Reference computation (per batch b, channel c, position (h,w)):
    q = W_q x ; k = W_k x_pad ; v = W_v x_pad           (1x1 convs)
    s[i,j]  = q[h,w] * (k[h+i, w+j] + bias[c, i or j])  over a 7x7 window
    out     = sum_ij softmax_ij(s) * v[h+i, w+j]

Sharding: spatial, 8 ways — core = (batch, 12-row slab). Fully independent
per core (no collectives). Host pre-pads each slab with the 3-row/col halo.

Per-core dataflow (channels on partitions, 2 channel-tiles of 128):
  TensorE : q/k/v GEMMs on fp16 inputs; den|num 49-tap reduction via
            identity-matmul accumulation into one 3-bank PSUM region
            [den(576) | num(576)] per tile.  The region is pre-zeroed by
            start=True matmuls of a zero tile (a start=True matmul marks
            its whole 2KB PSUM bank pending-zero, so the den-tail and
            num-head streams that share the middle bank must both
            accumulate with start=False onto pre-zeroed banks).
  ScalarE : exp over contiguous e-chunks (strided activation APs cost +29%
            on hw), plus PSUM evictions of the k/v maps.
  VectorE : score mults q*kb and most weight mults e*v (fp16/bf16 DVE 2x
            via shifted-window APs), biased-k stacks (4x tensor_scalar),
            q eviction, reciprocal.
  GpSimd  : tuned slice of the mults + the final divide.
  DMA     : kb1/v1 one-column-shifted copies (4B alignment for odd taps),
            batched one DMA per stack.

The repeat loop used by the benchmark emits bodies with parity-alternated
k/v/q buffers inside an unrolled For_i, so body n+1's head (input DMA,
GEMMs, stack production — all DVE-light) overlaps body n's attention
phase.  em tiles ride one shared ring across bodies.  Tile-0 kb slabs
hold only the 12-row window group g reads (rows g..g+11); tile-1 slabs
need all 18 rows (row-shifted taps) and stay full.

em layout per (group, tile): [e0..e6 (7x576 fp16 scores, exp'd in place
to bf16) | m0..m6 (bf16)], with mt0 slot order [j=0,2,4,6, j=1,3,5] so
each parity half is a contiguous exp chunk.
"""
import os
import numpy as np
from contextlib import ExitStack

from concourse import bass, bacc, mybir, tile
from concourse.bass_utils import run_bass_kernel_spmd

F32 = mybir.dt.float32
F16 = mybir.dt.float16
BF16 = mybir.dt.bfloat16

K, PAD = 7, 3
B, CIN, COUT, H, W = 2, 256, 256, 48, 48
ROWS = 12                 # output rows per core
SH, SW = ROWS + 2 * PAD, W + 2 * PAD   # 18, 54 padded slab
NPOS = ROWS * W           # 576 output positions per core
NPAD = SH * SW            # 972 padded positions
NQ = ROWS * SW            # 648 q-map positions
N_CORES = 8
EBLK = K * NPOS           # 4032: e-block (and m-block) width per group
NSLB = ROWS * SW + 8      # 656: windowed tile-0 kb slab (12 rows + margin)
NWIN = ROWS * SW          # 648 written cols per windowed slab

F32R = mybir.dt.float32r
GEMM_DT = os.environ.get("GEMM_DT", "f16")

# ---- engine assignment knobs (tuned on hw) ----
POOL_M0 = set(int(c) for c in os.environ.get("POOL_M0", ""))
POOL_S1 = set(int(c) for c in os.environ.get("POOL_S1", ""))
POOL_M1 = set()
for tok in os.environ.get("POOL_M1", "").split(","):
    if tok:
        POOL_M1.add((int(tok[0]), int(tok[1])))
POOL_DIV = bool(int(os.environ.get("POOL_DIV", "0")))  # GpSimd can't read PSUM
ACT_QEV = bool(int(os.environ.get("ACT_QEV", "1")))
E_BUFS = int(os.environ.get("E_BUFS", "7"))
M_BUFS = int(os.environ.get("M_BUFS", "3"))
# num-accum emitted NUM_DELAY groups behind its m-mult, so a slow (Pool)
# m-mult never stalls PE's in-order stream; needs M_BUFS >= 2 + NUM_DELAY
NUM_DELAY = int(os.environ.get("NUM_DELAY", "1"))
# delay the m-mults one group behind the scores, so DVE runs
# score(g+1) during exp(g) instead of stalling for it
M_DELAY = int(os.environ.get("M_DELAY", "1"))
MT_ILV = bool(int(os.environ.get("MT_ILV", "0")))
WARM_MM = int(os.environ.get("WARM_MM", "16"))
PAIR = bool(int(os.environ.get("PAIR", "1")))
UNROLL = int(os.environ.get("UNROLL", "8"))
STAGGER = bool(int(os.environ.get("STAGGER", "0")))
SPLIT_MT1 = bool(int(os.environ.get("SPLIT_MT1", "0")))
EXP1 = bool(int(os.environ.get("EXP1", "0")))  # one exp instr per group
POOL_STACK = bool(int(os.environ.get("POOL_STACK", "0")))
# stack as Pool tensor_tensor with stride-0 broadcast beta (Pool TS is
# ~10x slow but Pool TT is fine); off the critical chain (head phase)
STACK_TT = bool(int(os.environ.get("STACK_TT", "0")))
DIV_VIA = os.environ.get("DIV_VIA", "dve")  # dve | pool
SKIP = set(os.environ.get("SKIP", "").split(",")) - {""}

JEVEN = [0, 2, 4, 6]
JODD = [1, 3, 5]

_CACHED = {}


def _fap(t, offset, dims):
    """Custom free-dim AP on a tile: dims = [[stride, size], ...]."""
    a = t[:]
    return bass.AP(a.tensor, a.offset + offset, [list(a.ap[0])] + dims)


class _Ctx:
    pass


def _emit_prologue(nc, tc, stk):
    cx = _Ctx()
    cx.const = stk.enter_context(tc.tile_pool(name="const", bufs=1))
    cx.work = stk.enter_context(tc.tile_pool(name="work", bufs=1))
    cx.gpsum = stk.enter_context(
        tc.tile_pool(name="gpsum", bufs=1, space="PSUM"))
    cx.apsum = stk.enter_context(
        tc.tile_pool(name="apsum", bufs=1, space="PSUM"))
    cx.ring = stk.enter_context(tc.tile_pool(name="ring", bufs=2))

    GDT = {"f16": F16, "f32r": F32R, "f32": F32}[GEMM_DT]
    cx.gdt = GDT
    # beta(2K) and identity(128) share one f32 dram tensor / one DMA
    cx.bi = cx.const.tile([128, 2 * K + 128], F32, name="bi")
    cx.idb = cx.const.tile([128, 128], BF16, name="idb")
    cx.zz = cx.const.tile([128, 512], BF16, name="zz")    # PSUM pre-zero src
    cx.x_sb = [cx.const.tile([128, NPAD], GDT, name=f"x_sb{kt}")
               for kt in range(2)]
    # w layout: [kt][k|q|v][mt] blocks of 128 cols -> [128, 2*3*2*128]
    cx.w_sb = cx.const.tile([128, 12 * 128], GDT, name="w_sb")
    cx.warm = cx.const.tile([128, 2], F32, name="warm")
    cx.k0 = [cx.work.tile([128, NPAD], F16, name=f"k0_{mt}")
             for mt in range(2)]
    cx.kb0 = {}
    cx.kb1 = {}
    cx.v0 = {}
    cx.v1 = {}
    cx.q_sb = {}
    for p in range(2 if PAIR else 1):
        cx.kb0[p, 0] = cx.work.tile([128, K * NSLB], F16, name=f"kb0_{p}0")
        cx.kb0[p, 1] = cx.work.tile([128, K * NPAD], F16, name=f"kb0_{p}1")
        cx.kb1[p, 0] = cx.work.tile([128, K * NSLB], F16, name=f"kb1_{p}0")
        cx.kb1[p, 1] = cx.work.tile([128, 3 * NPAD], F16, name=f"kb1_{p}1")
        for mt in range(2):
            cx.v0[p, mt] = cx.work.tile([128, NPAD], F16, name=f"v0_{p}{mt}")
            cx.v1[p, mt] = cx.work.tile([128, NPAD], F16, name=f"v1_{p}{mt}")
            cx.q_sb[p, mt] = cx.work.tile([128, NPOS], F16, name=f"q_{p}{mt}")
    # dn = [den(576) | num(576)] f32 = 3 PSUM banks per channel tile
    cx.dn = [cx.apsum.tile([128, 2 * NPOS], F32, name=f"dn{mt}")
             for mt in range(2)]
    return cx


def _emit_body(nc, tc, cx, dram, par_i, warm=False):
    x_d, w_d, bi_d, out_d = dram
    MULT = mybir.AluOpType.mult
    p = par_i if PAIR else 0
    cx.body_n = getattr(cx, "body_n", -1) + 1
    bn = cx.body_n
    beta = cx.bi       # beta columns [0 : 2K]
    kb0 = [cx.kb0[p, mt] for mt in range(2)]
    kb1 = [cx.kb1[p, mt] for mt in range(2)]
    v0 = [cx.v0[p, mt] for mt in range(2)]
    v1 = [cx.v1[p, mt] for mt in range(2)]
    q_sb = [cx.q_sb[p, mt] for mt in range(2)]

    # ---- input DMAs (batched: 4 loads) ----
    if "dma" not in SKIP:
        nc.sync.dma_start(cx.bi[:], bi_d[:, :])
        for kt in range(2):
            nc.sync.dma_start(cx.x_sb[kt][:],
                              x_d[kt * 128:(kt + 1) * 128, :])
        nc.sync.dma_start(cx.w_sb[:], w_d[:, :])
    if warm:
        nc.vector.tensor_copy(cx.idb[:], cx.bi[:, 2 * K:])
        nc.vector.memset(cx.zz[:], 0.0)
        nc.scalar.activation(cx.warm[:], cx.bi[:, :2],
                             mybir.ActivationFunctionType.Exp)
        if WARM_MM:
            for wi in range(WARM_MM):
                nc.tensor.matmul(cx.dn[0][:, :128], cx.idb[:], cx.idb[:],
                                 start=(wi == 0), stop=(wi == WARM_MM - 1))

    def wsl(nm, kt, mt):
        base = ((kt * 3) + {"k": 0, "q": 1, "v": 2}[nm]) * 2 + mt
        return cx.w_sb[:, base * 128:(base + 1) * 128]

    # ---- GEMMs + map production ----
    for mt in range(2) if "gemm" not in SKIP else ():
        # k map
        kp = cx.gpsum.tile([128, NPAD], F32, tag="gp", bufs=1, name=f"kp{mt}")
        for kt in range(2):
            for c0, c1 in ((0, 512), (512, NPAD)):
                nc.tensor.matmul(kp[:, c0:c1], wsl("k", kt, mt),
                                 cx.x_sb[kt][:, c0:c1],
                                 start=(kt == 0), stop=(kt == 1))
        nc.scalar.copy(cx.k0[mt][:], kp[:])
        # biased stacks: fp16 tensor_scalar adds run the 4x DVE mode
        if "stack" not in SKIP:
            ADD = mybir.AluOpType.add
            for t in range(K):
                if STACK_TT:
                    bt = t if mt == 0 else K + t
                    nw = NWIN if mt == 0 else NPAD
                    src_ap = (cx.k0[0][:, t * SW:t * SW + NWIN] if mt == 0
                              else cx.k0[1][:])
                    nc.gpsimd.tensor_tensor(
                        kb0[mt][:, t * (NSLB if mt == 0 else NPAD):
                                t * (NSLB if mt == 0 else NPAD) + nw],
                        src_ap, _fap(cx.bi, bt, [[0, nw]]), ADD)
                elif mt == 0:
                    nc.vector.tensor_scalar_add(
                        kb0[0][:, t * NSLB:t * NSLB + NWIN],
                        cx.k0[0][:, t * SW:t * SW + NWIN],
                        beta[:, t:t + 1])
                else:
                    nc.vector.tensor_scalar_add(
                        kb0[1][:, t * NPAD:(t + 1) * NPAD], cx.k0[1][:],
                        beta[:, K + t:K + t + 1])
            # 1-col-shifted copies, one batched DMA per stack
            if mt == 0:
                nc.sync.dma_start(
                    _fap(kb1[0], 0, [[NSLB, K], [1, NWIN - 1]]),
                    _fap(kb0[0], 1, [[NSLB, K], [1, NWIN - 1]]))
            else:
                nc.sync.dma_start(
                    _fap(kb1[1], 0, [[NPAD, 3], [1, NPAD - 2]]),
                    _fap(kb0[1], NPAD + 1, [[2 * NPAD, 3], [1, NPAD - 2]]))
        # q map: only the 12 center rows (cols incl. pad)
        qp = cx.gpsum.tile([128, NQ], F32, tag="gp", bufs=1, name=f"qp{mt}")
        for kt in range(2):
            for c0, c1 in ((0, 512), (512, NQ)):
                nc.tensor.matmul(qp[:, c0:c1], wsl("q", kt, mt),
                                 cx.x_sb[kt][:, PAD * SW + c0:PAD * SW + c1],
                                 start=(kt == 0), stop=(kt == 1))
        qsrc = _fap(qp, PAD, [[SW, ROWS], [1, W]])
        qdst = q_sb[mt][:].rearrange("p (h w) -> p h w", h=ROWS)
        if ACT_QEV:
            nc.scalar.copy(qdst, qsrc)
        else:
            nc.vector.tensor_copy(qdst, qsrc)
        # v map
        vp = cx.gpsum.tile([128, NPAD], F32, tag="gp", bufs=1, name=f"vp{mt}")
        for kt in range(2):
            for c0, c1 in ((0, 512), (512, NPAD)):
                nc.tensor.matmul(vp[:, c0:c1], wsl("v", kt, mt),
                                 cx.x_sb[kt][:, c0:c1],
                                 start=(kt == 0), stop=(kt == 1))
        nc.scalar.copy(v0[mt][:], vp[:])
        nc.sync.dma_start(v1[mt][:, :NPAD - 2], v0[mt][:, 1:NPAD - 1])

    # ---- attention ----
    # pre-zero dn via start=True matmuls of the zero tile (marks+clears the
    # 3 banks); all real accumulation runs start=False on top.
    for mt in range(2):
        for c0, c1 in ((0, 512), (512, 1024), (1024, 2 * NPOS)):
            nc.tensor.matmul(cx.dn[mt][:, c0:c1], cx.idb[:],
                             cx.zz[:, :c1 - c0], start=True, stop=True,
                             skip_group_check=True)
    order = ([(g, mt) for g in range(K) for mt in range(2)] if MT_ILV
             else [(g, mt) for mt in range(2) for g in range(K)])
    done = set()
    pend_num = []
    pend_m = []
    for g, mt in order:
        # separate e/m ring tiles: e_t frees after (m-mult, den-accum), m_t
        # after num-accum — independent release deepens the pipeline
        e_t = cx.ring.tile([128, EBLK], BF16, tag="e", bufs=E_BUFS,
                           name=f"e{mt}_{g}_{bn}")
        m_t = cx.ring.tile([128, EBLK], BF16, tag="m", bufs=M_BUFS,
                           name=f"m{mt}_{g}_{bn}")
        last = (g == K - 1)

        def score(half, e_t=e_t, mt=mt, g=g):
            if mt == 0:
                # groups = j (col shift, parity via kb1), slots = i via the
                # slab stride: one aligned instr covers all 7 i-taps
                par = g % 2
                if SPLIT_MT1:
                    i0, ni = ((0, 4), (4, 3))[half]
                elif half:
                    return
                else:
                    i0, ni = 0, K
                kb_ap = _fap(kb1[0] if par else kb0[0],
                             (g - par) + i0 * NSLB,
                             [[NSLB, ni], [SW, ROWS], [1, W]])
                q_ap = _fap(q_sb[0], 0, [[0, ni], [W, ROWS], [1, W]])
                s_ap = _fap(e_t, i0 * NPOS,
                            [[NPOS, ni], [W, ROWS], [1, W]]).bitcast(F16)
                nc.vector.tensor_tensor(s_ap, kb_ap, q_ap, MULT)
            else:
                par = g % 2
                if par:
                    kb_t = kb1[1]
                    bb = ((g - 1) // 2) * NPAD + (g - 1)
                else:
                    kb_t, bb = kb0[1], g * NPAD + g
                if SPLIT_MT1:
                    i0, ni = ((0, 4), (4, 3))[half]
                elif half:
                    return
                else:
                    i0, ni = 0, K
                kb_ap = _fap(kb_t, bb + i0 * SW,
                             [[SW, ni], [SW, ROWS], [1, W]])
                q_ap = _fap(q_sb[1], 0, [[0, ni], [W, ROWS], [1, W]])
                s_ap = _fap(e_t, i0 * NPOS,
                            [[NPOS, ni], [W, ROWS], [1, W]]).bitcast(F16)
                eng = nc.gpsimd if g in POOL_S1 else nc.vector
                eng.tensor_tensor(s_ap, kb_ap, q_ap, MULT)

        def expf(half, e_t=e_t):
            if EXP1:
                if half:
                    return
                s0, s1 = 0, EBLK
            else:
                s0, s1 = ((0, 4 * NPOS), (4 * NPOS, EBLK))[half]
            nc.scalar.activation(
                _fap(e_t, s0, [[1, s1 - s0]]),
                _fap(e_t, s0, [[1, s1 - s0]]).bitcast(F16),
                mybir.ActivationFunctionType.Exp)

        def mmul(half, e_t=e_t, m_t=m_t, mt=mt, g=g):
            if mt == 0:
                par = g % 2
                if SPLIT_MT1:
                    i0, ni = ((0, 4), (4, 3))[half]
                elif half:
                    return
                else:
                    i0, ni = 0, K
                e_ap = _fap(e_t, i0 * NPOS, [[NPOS, ni], [W, ROWS], [1, W]])
                v_ap = _fap(v1[0] if par else v0[0],
                            (g - par) + i0 * SW,
                            [[SW, ni], [SW, ROWS], [1, W]])
                m_ap = _fap(m_t, i0 * NPOS,
                            [[NPOS, ni], [W, ROWS], [1, W]])
                eng = (nc.gpsimd if g in POOL_M0 else nc.vector)
                eng.tensor_tensor(m_ap, e_ap, v_ap, MULT)
            else:
                par = g % 2
                if SPLIT_MT1:
                    i0, ni = ((0, 4), (4, 3))[half]
                elif half:
                    return
                else:
                    i0, ni = 0, K
                e_ap = _fap(e_t, i0 * NPOS, [[NPOS, ni], [W, ROWS], [1, W]])
                v_ap = _fap(v1[1] if par else v0[1],
                            g - par + i0 * SW,
                            [[SW, ni], [SW, ROWS], [1, W]])
                m_ap = _fap(m_t, i0 * NPOS,
                            [[NPOS, ni], [W, ROWS], [1, W]])
                eng = (nc.gpsimd if (g, half) in POOL_M1 else nc.vector)
                eng.tensor_tensor(m_ap, e_ap, v_ap, MULT)

        def accum(half, blk, e_t=e_t, m_t=m_t, mt=mt, last=last):
            # bind tiles at definition: the closure is queued in pend_num
            # and must keep THIS group's tiles, not the loop's latest.
            # chunks must stay within 512-col PSUM banks:
            # den at dn[0:576]   -> (0,512),(512,576)
            # num at dn[576:1152] -> (576,1024),(1024,1152)
            off = 0 if blk == 0 else NPOS
            chunks = ((0, 512), (512, NPOS)) if blk == 0 else \
                     ((0, 448), (448, NPOS))
            sls = (range(0, 4), range(4, K))[half]
            src_t = e_t if blk == 0 else m_t
            for sl in sls:
                for c0, c1 in chunks:
                    nc.tensor.matmul(
                        cx.dn[mt][:, off + c0:off + c1], cx.idb[:],
                        src_t[:, sl * NPOS + c0:sl * NPOS + c1],
                        start=False,
                        stop=(last and sl == K - 1 and half == 1),
                        skip_group_check=True)

        if "score" not in SKIP:
            score(0)
            score(1)
        if "exp" not in SKIP:
            expf(0)
            expf(1)
        if M_DELAY == 0:
            if "mmul" not in SKIP:
                mmul(0)
            if "accum" not in SKIP:
                accum(0, 0)
            if "mmul" not in SKIP:
                mmul(1)
            if "accum" not in SKIP:
                if NUM_DELAY == 0:
                    accum(0, EBLK)
                    accum(1, 0)
                    accum(1, EBLK)
                else:
                    accum(1, 0)
                    pend_num.append((accum, mt))
                    while len(pend_num) > NUM_DELAY:
                        fn, _ = pend_num.pop(0)
                        fn(0, EBLK)
                        fn(1, EBLK)
        else:
            if "accum" not in SKIP:
                accum(0, 0)
                accum(1, 0)
            pend_m.append((mmul, accum, mt))
            while len(pend_m) > M_DELAY:
                mf, af, _ = pend_m.pop(0)
                if "mmul" not in SKIP:
                    mf(0)
                if "accum" not in SKIP:
                    af(0, EBLK)
                if "mmul" not in SKIP:
                    mf(1)
                if "accum" not in SKIP:
                    af(1, EBLK)
        done.add((g, mt))
        if "out" in SKIP:
            continue
        if all((gg, mt) in done for gg in range(K)):
            while pend_m and pend_m[0][2] == mt:
                mf, af, _ = pend_m.pop(0)
                if "mmul" not in SKIP:
                    mf(0)
                if "accum" not in SKIP:
                    af(0, EBLK)
                if "mmul" not in SKIP:
                    mf(1)
                if "accum" not in SKIP:
                    af(1, EBLK)
            while pend_num and pend_num[0][1] == mt:
                fn, _ = pend_num.pop(0)
                fn(0, EBLK)
                fn(1, EBLK)
            rden = cx.ring.tile([128, NPOS], F32, tag="rden", bufs=2,
                                name=f"rden{mt}_{bn}")
            nc.vector.reciprocal(rden[:], cx.dn[mt][:, :NPOS])
            o_t = cx.ring.tile([128, NPOS], F32, tag="o", bufs=2,
                               name=f"o{mt}_{bn}")
            if DIV_VIA == "pool":
                # evict num to SBUF on ACT, divide on Pool (can't read PSUM)
                nev = cx.ring.tile([128, NPOS], F32, tag="nev", bufs=2,
                                   name=f"nev{mt}_{bn}")
                nc.scalar.copy(nev[:], cx.dn[mt][:, NPOS:])
                for c0, c1 in ((0, NPOS // 2), (NPOS // 2, NPOS)):
                    nc.gpsimd.tensor_tensor(o_t[:, c0:c1], nev[:, c0:c1],
                                            rden[:, c0:c1], MULT)
                    nc.sync.dma_start(out_d[mt * 128:(mt + 1) * 128, c0:c1],
                                      o_t[:, c0:c1])
            else:
                for c0, c1 in ((0, NPOS // 2), (NPOS // 2, NPOS)):
                    nc.vector.tensor_tensor(o_t[:, c0:c1],
                                            cx.dn[mt][:, NPOS + c0:NPOS + c1],
                                            rden[:, c0:c1], MULT)
                    nc.sync.dma_start(out_d[mt * 128:(mt + 1) * 128, c0:c1],
                                      o_t[:, c0:c1])


def _build_graph(repeat=1):
    nc = bacc.Bacc("TRN2", target_bir_lowering=False, debug=False,
                   num_devices=N_CORES)

    GDT = {"f16": F16, "f32r": F32R, "f32": F32}[GEMM_DT]
    dram = (
        nc.declare_dram_parameter("x_slab", [CIN, NPAD], GDT, isOutput=False),
        nc.declare_dram_parameter("w_pack", [128, 12 * 128], GDT,
                                  isOutput=False),
        nc.declare_dram_parameter("bi_pack", [128, 2 * K + 128], F32,
                                  isOutput=False),
        nc.declare_dram_parameter("out", [COUT, NPOS], F32, isOutput=True),
    )

    with tile.TileContext(nc) as tc:
        with ExitStack() as stk:
            cx = _emit_prologue(nc, tc, stk)
            _emit_body(nc, tc, cx, dram, 0, warm=True)
            left = repeat - 1
            if left > 0:
                u = min(UNROLL, left)
                trips, rem = divmod(left, u)
                if trips > 0:
                    with tc.For_i(0, trips, 1, staggered_reset=STAGGER):
                        for j in range(u):
                            _emit_body(nc, tc, cx, dram, (j + 1) % 2)
                for j in range(rem):
                    _emit_body(nc, tc, cx, dram, (j + 1) % 2)

    nc.compile()
    return nc


def _prep_host(x, w_q, w_k, w_v, rel_h, rel_w):
    gnp = {"f16": np.float16, "f32r": np.float32,
           "f32": np.float32}[GEMM_DT]
    x = np.ascontiguousarray(x, np.float32)
    beta = np.zeros((COUT, K), np.float32)
    beta[:COUT // 2] = rel_h.reshape(COUT // 2, K)
    beta[COUT // 2:] = rel_w.reshape(COUT // 2, K)
    bi_pack = np.empty((128, 2 * K + 128), np.float32)
    for mt in range(2):
        bi_pack[:, mt * K:(mt + 1) * K] = beta[mt * 128:(mt + 1) * 128]
    bi_pack[:, 2 * K:] = np.eye(128, dtype=np.float32)
    # w layout: [kt][k|q|v][mt] blocks of 128 cols
    w_pack = np.empty((128, 12 * 128), gnp)
    ws = {"k": w_k, "q": w_q, "v": w_v}
    for kt in range(2):
        for wi, nm in enumerate("kqv"):
            for mt in range(2):
                blk = ((kt * 3) + wi) * 2 + mt
                w_pack[:, blk * 128:(blk + 1) * 128] = \
                    ws[nm].T[kt * 128:(kt + 1) * 128,
                             mt * 128:(mt + 1) * 128].astype(gnp)
    common = {"w_pack": w_pack, "bi_pack": bi_pack}
    in_maps = []
    for core in range(N_CORES):
        b, r0 = divmod(core, 4)
        r0 *= ROWS
        slab = np.zeros((CIN, SH, SW), np.float32)
        lo, hi = r0 - PAD, r0 + ROWS + PAD
        clo, chi = max(lo, 0), min(hi, H)
        slab[:, clo - lo:chi - lo, PAD:PAD + W] = x[b, :, clo:chi, :]
        in_maps.append({"x_slab": slab.reshape(CIN, NPAD).astype(gnp),
                        **common})
    return in_maps


def kernel(x, w_q, w_k, w_v, rel_h, rel_w):
    if "nc" not in _CACHED:
        _CACHED["nc"] = _build_graph()
    nc = _CACHED["nc"]
    in_maps = _prep_host(x, w_q, w_k, w_v, rel_h, rel_w)
    res = run_bass_kernel_spmd(nc, in_maps, core_ids=list(range(N_CORES)))
    _CACHED["exec_time_ns"] = res.exec_time_ns
    out = np.empty((B, COUT, H, W), np.float32)
    for core in range(N_CORES):
        b, r0 = divmod(core, 4)
        r0 *= ROWS
        out[b, :, r0:r0 + ROWS, :] = \
            res.results[core]["out"].reshape(COUT, ROWS, W)
    return out


# revision 38
# speedup vs baseline: 1.2217x; 1.0040x over previous
"""Trainium2 Bass kernel for local windowed per-channel attention (sparse_attention).

Reference computation (per batch b, channel c, position (h,w)):
    q = W_q x ; k = W_k x_pad ; v = W_v x_pad           (1x1 convs)
    s[i,j]  = q[h,w] * (k[h+i, w+j] + bias[c, i or j])  over a 7x7 window
    out     = sum_ij softmax_ij(s) * v[h+i, w+j]

Sharding: spatial, 8 ways — core = (batch, 12-row slab). Fully independent
per core (no collectives). Host pre-pads each slab with the 3-row/col halo.

Per-core dataflow (channels on partitions, 2 channel-tiles of 128):
  TensorE : q/k/v GEMMs on fp16 inputs; den|num 49-tap reduction via
            identity-matmul accumulation into one 3-bank PSUM region
            [den(576) | num(576)] per tile.  The region is pre-zeroed by
            start=True matmuls of a zero tile (a start=True matmul marks
            its whole 2KB PSUM bank pending-zero, so the den-tail and
            num-head streams that share the middle bank must both
            accumulate with start=False onto pre-zeroed banks).
  ScalarE : exp over contiguous e-chunks (strided activation APs cost +29%
            on hw), plus PSUM evictions of the k/v maps.
  VectorE : score mults q*kb and most weight mults e*v (fp16/bf16 DVE 2x
            via shifted-window APs), biased-k stacks (4x tensor_scalar),
            q eviction, reciprocal.
  GpSimd  : tuned slice of the mults + the final divide.
  DMA     : kb1/v1 one-column-shifted copies (4B alignment for odd taps),
            batched one DMA per stack.

The repeat loop used by the benchmark emits bodies with parity-alternated
k/v/q buffers inside an unrolled For_i, so body n+1's head (input DMA,
GEMMs, stack production — all DVE-light) overlaps body n's attention
phase.  em tiles ride one shared ring across bodies.  Tile-0 kb slabs
hold only the 12-row window group g reads (rows g..g+11); tile-1 slabs
need all 18 rows (row-shifted taps) and stay full.

em layout per (group, tile): [e0..e6 (7x576 fp16 scores, exp'd in place
to bf16) | m0..m6 (bf16)], with mt0 slot order [j=0,2,4,6, j=1,3,5] so
each parity half is a contiguous exp chunk.
"""
import os
import numpy as np
from contextlib import ExitStack

from concourse import bass, bacc, mybir, tile
from concourse.bass_utils import run_bass_kernel_spmd

F32 = mybir.dt.float32
F16 = mybir.dt.float16
BF16 = mybir.dt.bfloat16

K, PAD = 7, 3
B, CIN, COUT, H, W = 2, 256, 256, 48, 48
ROWS = 12                 # output rows per core
SH, SW = ROWS + 2 * PAD, W + 2 * PAD   # 18, 54 padded slab
NPOS = ROWS * W           # 576 output positions per core
NPAD = SH * SW            # 972 padded positions
NQ = ROWS * SW            # 648 q-map positions
N_CORES = 8
EBLK = K * NPOS           # 4032: e-block (and m-block) width per group
NSLB = ROWS * SW + 8      # 656: windowed tile-0 kb slab (12 rows + margin)
NWIN = ROWS * SW          # 648 written cols per windowed slab

F32R = mybir.dt.float32r
GEMM_DT = os.environ.get("GEMM_DT", "f16")

# ---- engine assignment knobs (tuned on hw) ----
POOL_M0 = set(int(c) for c in os.environ.get("POOL_M0", ""))
POOL_S1 = set(int(c) for c in os.environ.get("POOL_S1", ""))
POOL_M1 = set()
for tok in os.environ.get("POOL_M1", "").split(","):
    if tok:
        POOL_M1.add((int(tok[0]), int(tok[1])))
POOL_DIV = bool(int(os.environ.get("POOL_DIV", "0")))  # GpSimd can't read PSUM
ACT_QEV = bool(int(os.environ.get("ACT_QEV", "1")))
E_BUFS = int(os.environ.get("E_BUFS", "7"))
M_BUFS = int(os.environ.get("M_BUFS", "3"))
# num-accum emitted NUM_DELAY groups behind its m-mult, so a slow (Pool)
# m-mult never stalls PE's in-order stream; needs M_BUFS >= 2 + NUM_DELAY
NUM_DELAY = int(os.environ.get("NUM_DELAY", "1"))
# delay the m-mults one group behind the scores, so DVE runs
# score(g+1) during exp(g) instead of stalling for it
M_DELAY = int(os.environ.get("M_DELAY", "1"))
MT_ILV = bool(int(os.environ.get("MT_ILV", "0")))
WARM_MM = int(os.environ.get("WARM_MM", "16"))
PAIR = bool(int(os.environ.get("PAIR", "1")))
UNROLL = int(os.environ.get("UNROLL", "8"))
STAGGER = bool(int(os.environ.get("STAGGER", "0")))
SPLIT_MT1 = bool(int(os.environ.get("SPLIT_MT1", "0")))
EXP1 = bool(int(os.environ.get("EXP1", "0")))  # one exp instr per group
POOL_STACK = bool(int(os.environ.get("POOL_STACK", "0")))
# stack as Pool tensor_tensor with stride-0 broadcast beta (Pool TS is
# ~10x slow but Pool TT is fine); off the critical chain (head phase)
STACK_TT = bool(int(os.environ.get("STACK_TT", "0")))
DIV_VIA = os.environ.get("DIV_VIA", "dve")  # dve | pool
DIV1 = bool(int(os.environ.get("DIV1", "1")))  # single-chunk divide
SKIP = set(os.environ.get("SKIP", "").split(",")) - {""}

JEVEN = [0, 2, 4, 6]
JODD = [1, 3, 5]

_CACHED = {}


def _fap(t, offset, dims):
    """Custom free-dim AP on a tile: dims = [[stride, size], ...]."""
    a = t[:]
    return bass.AP(a.tensor, a.offset + offset, [list(a.ap[0])] + dims)


class _Ctx:
    pass


def _emit_prologue(nc, tc, stk):
    cx = _Ctx()
    cx.const = stk.enter_context(tc.tile_pool(name="const", bufs=1))
    cx.work = stk.enter_context(tc.tile_pool(name="work", bufs=1))
    cx.gpsum = stk.enter_context(
        tc.tile_pool(name="gpsum", bufs=1, space="PSUM"))
    cx.apsum = stk.enter_context(
        tc.tile_pool(name="apsum", bufs=1, space="PSUM"))
    cx.ring = stk.enter_context(tc.tile_pool(name="ring", bufs=2))

    GDT = {"f16": F16, "f32r": F32R, "f32": F32}[GEMM_DT]
    cx.gdt = GDT
    # beta(2K) and identity(128) share one f32 dram tensor / one DMA
    cx.bi = cx.const.tile([128, 2 * K + 128], F32, name="bi")
    cx.idb = cx.const.tile([128, 128], BF16, name="idb")
    cx.zz = cx.const.tile([128, 512], BF16, name="zz")    # PSUM pre-zero src
    cx.x_sb = [cx.const.tile([128, NPAD], GDT, name=f"x_sb{kt}")
               for kt in range(2)]
    # w layout: [kt][k|q|v][mt] blocks of 128 cols -> [128, 2*3*2*128]
    cx.w_sb = cx.const.tile([128, 12 * 128], GDT, name="w_sb")
    cx.warm = cx.const.tile([128, 2], F32, name="warm")
    cx.k0 = [cx.work.tile([128, NPAD], F16, name=f"k0_{mt}")
             for mt in range(2)]
    cx.kb0 = {}
    cx.kb1 = {}
    cx.v0 = {}
    cx.v1 = {}
    cx.q_sb = {}
    for p in range(2 if PAIR else 1):
        cx.kb0[p, 0] = cx.work.tile([128, K * NSLB], F16, name=f"kb0_{p}0")
        cx.kb0[p, 1] = cx.work.tile([128, K * NPAD], F16, name=f"kb0_{p}1")
        cx.kb1[p, 0] = cx.work.tile([128, K * NSLB], F16, name=f"kb1_{p}0")
        cx.kb1[p, 1] = cx.work.tile([128, 3 * NPAD], F16, name=f"kb1_{p}1")
        for mt in range(2):
            cx.v0[p, mt] = cx.work.tile([128, NPAD], F16, name=f"v0_{p}{mt}")
            cx.v1[p, mt] = cx.work.tile([128, NPAD], F16, name=f"v1_{p}{mt}")
            cx.q_sb[p, mt] = cx.work.tile([128, NPOS], F16, name=f"q_{p}{mt}")
    # dn = [den(576) | num(576)] f32 = 3 PSUM banks per channel tile
    cx.dn = [cx.apsum.tile([128, 2 * NPOS], F32, name=f"dn{mt}")
             for mt in range(2)]
    return cx


def _emit_body(nc, tc, cx, dram, par_i, warm=False):
    x_d, w_d, bi_d, out_d = dram
    MULT = mybir.AluOpType.mult
    p = par_i if PAIR else 0
    cx.body_n = getattr(cx, "body_n", -1) + 1
    bn = cx.body_n
    beta = cx.bi       # beta columns [0 : 2K]
    kb0 = [cx.kb0[p, mt] for mt in range(2)]
    kb1 = [cx.kb1[p, mt] for mt in range(2)]
    v0 = [cx.v0[p, mt] for mt in range(2)]
    v1 = [cx.v1[p, mt] for mt in range(2)]
    q_sb = [cx.q_sb[p, mt] for mt in range(2)]

    # ---- input DMAs (batched: 4 loads) ----
    if "dma" not in SKIP:
        nc.sync.dma_start(cx.bi[:], bi_d[:, :])
        for kt in range(2):
            nc.sync.dma_start(cx.x_sb[kt][:],
                              x_d[kt * 128:(kt + 1) * 128, :])
        nc.sync.dma_start(cx.w_sb[:], w_d[:, :])
    if warm:
        nc.vector.tensor_copy(cx.idb[:], cx.bi[:, 2 * K:])
        nc.vector.memset(cx.zz[:], 0.0)
        nc.scalar.activation(cx.warm[:], cx.bi[:, :2],
                             mybir.ActivationFunctionType.Exp)
        if WARM_MM:
            for wi in range(WARM_MM):
                nc.tensor.matmul(cx.dn[0][:, :128], cx.idb[:], cx.idb[:],
                                 start=(wi == 0), stop=(wi == WARM_MM - 1))

    def wsl(nm, kt, mt):
        base = ((kt * 3) + {"k": 0, "q": 1, "v": 2}[nm]) * 2 + mt
        return cx.w_sb[:, base * 128:(base + 1) * 128]

    # ---- GEMMs + map production ----
    for mt in range(2) if "gemm" not in SKIP else ():
        # k map
        kp = cx.gpsum.tile([128, NPAD], F32, tag="gp", bufs=1, name=f"kp{mt}")
        for kt in range(2):
            for c0, c1 in ((0, 512), (512, NPAD)):
                nc.tensor.matmul(kp[:, c0:c1], wsl("k", kt, mt),
                                 cx.x_sb[kt][:, c0:c1],
                                 start=(kt == 0), stop=(kt == 1))
        nc.scalar.copy(cx.k0[mt][:], kp[:])
        # biased stacks: fp16 tensor_scalar adds run the 4x DVE mode
        if "stack" not in SKIP:
            ADD = mybir.AluOpType.add
            for t in range(K):
                if STACK_TT:
                    bt = t if mt == 0 else K + t
                    nw = NWIN if mt == 0 else NPAD
                    src_ap = (cx.k0[0][:, t * SW:t * SW + NWIN] if mt == 0
                              else cx.k0[1][:])
                    nc.gpsimd.tensor_tensor(
                        kb0[mt][:, t * (NSLB if mt == 0 else NPAD):
                                t * (NSLB if mt == 0 else NPAD) + nw],
                        src_ap, _fap(cx.bi, bt, [[0, nw]]), ADD)
                elif mt == 0:
                    nc.vector.tensor_scalar_add(
                        kb0[0][:, t * NSLB:t * NSLB + NWIN],
                        cx.k0[0][:, t * SW:t * SW + NWIN],
                        beta[:, t:t + 1])
                else:
                    nc.vector.tensor_scalar_add(
                        kb0[1][:, t * NPAD:(t + 1) * NPAD], cx.k0[1][:],
                        beta[:, K + t:K + t + 1])
            # 1-col-shifted copies, one batched DMA per stack
            if mt == 0:
                nc.sync.dma_start(
                    _fap(kb1[0], 0, [[NSLB, K], [1, NWIN - 1]]),
                    _fap(kb0[0], 1, [[NSLB, K], [1, NWIN - 1]]))
            else:
                nc.sync.dma_start(
                    _fap(kb1[1], 0, [[NPAD, 3], [1, NPAD - 2]]),
                    _fap(kb0[1], NPAD + 1, [[2 * NPAD, 3], [1, NPAD - 2]]))
        # q map: only the 12 center rows (cols incl. pad)
        qp = cx.gpsum.tile([128, NQ], F32, tag="gp", bufs=1, name=f"qp{mt}")
        for kt in range(2):
            for c0, c1 in ((0, 512), (512, NQ)):
                nc.tensor.matmul(qp[:, c0:c1], wsl("q", kt, mt),
                                 cx.x_sb[kt][:, PAD * SW + c0:PAD * SW + c1],
                                 start=(kt == 0), stop=(kt == 1))
        qsrc = _fap(qp, PAD, [[SW, ROWS], [1, W]])
        qdst = q_sb[mt][:].rearrange("p (h w) -> p h w", h=ROWS)
        if ACT_QEV:
            nc.scalar.copy(qdst, qsrc)
        else:
            nc.vector.tensor_copy(qdst, qsrc)
        # v map
        vp = cx.gpsum.tile([128, NPAD], F32, tag="gp", bufs=1, name=f"vp{mt}")
        for kt in range(2):
            for c0, c1 in ((0, 512), (512, NPAD)):
                nc.tensor.matmul(vp[:, c0:c1], wsl("v", kt, mt),
                                 cx.x_sb[kt][:, c0:c1],
                                 start=(kt == 0), stop=(kt == 1))
        nc.scalar.copy(v0[mt][:], vp[:])
        nc.sync.dma_start(v1[mt][:, :NPAD - 2], v0[mt][:, 1:NPAD - 1])

    # ---- attention ----
    # pre-zero dn via start=True matmuls of the zero tile (marks+clears the
    # 3 banks); all real accumulation runs start=False on top.
    for mt in range(2):
        for c0, c1 in ((0, 512), (512, 1024), (1024, 2 * NPOS)):
            nc.tensor.matmul(cx.dn[mt][:, c0:c1], cx.idb[:],
                             cx.zz[:, :c1 - c0], start=True, stop=True,
                             skip_group_check=True)
    order = ([(g, mt) for g in range(K) for mt in range(2)] if MT_ILV
             else [(g, mt) for mt in range(2) for g in range(K)])
    done = set()
    pend_num = []
    pend_m = []
    for g, mt in order:
        # separate e/m ring tiles: e_t frees after (m-mult, den-accum), m_t
        # after num-accum — independent release deepens the pipeline
        e_t = cx.ring.tile([128, EBLK], BF16, tag="e", bufs=E_BUFS,
                           name=f"e{mt}_{g}_{bn}")
        m_t = cx.ring.tile([128, EBLK], BF16, tag="m", bufs=M_BUFS,
                           name=f"m{mt}_{g}_{bn}")
        last = (g == K - 1)

        def score(half, e_t=e_t, mt=mt, g=g):
            if mt == 0:
                # groups = j (col shift, parity via kb1), slots = i via the
                # slab stride: one aligned instr covers all 7 i-taps
                par = g % 2
                if SPLIT_MT1:
                    i0, ni = ((0, 4), (4, 3))[half]
                elif half:
                    return
                else:
                    i0, ni = 0, K
                kb_ap = _fap(kb1[0] if par else kb0[0],
                             (g - par) + i0 * NSLB,
                             [[NSLB, ni], [SW, ROWS], [1, W]])
                q_ap = _fap(q_sb[0], 0, [[0, ni], [W, ROWS], [1, W]])
                s_ap = _fap(e_t, i0 * NPOS,
                            [[NPOS, ni], [W, ROWS], [1, W]]).bitcast(F16)
                nc.vector.tensor_tensor(s_ap, kb_ap, q_ap, MULT)
            else:
                par = g % 2
                if par:
                    kb_t = kb1[1]
                    bb = ((g - 1) // 2) * NPAD + (g - 1)
                else:
                    kb_t, bb = kb0[1], g * NPAD + g
                if SPLIT_MT1:
                    i0, ni = ((0, 4), (4, 3))[half]
                elif half:
                    return
                else:
                    i0, ni = 0, K
                kb_ap = _fap(kb_t, bb + i0 * SW,
                             [[SW, ni], [SW, ROWS], [1, W]])
                q_ap = _fap(q_sb[1], 0, [[0, ni], [W, ROWS], [1, W]])
                s_ap = _fap(e_t, i0 * NPOS,
                            [[NPOS, ni], [W, ROWS], [1, W]]).bitcast(F16)
                eng = nc.gpsimd if g in POOL_S1 else nc.vector
                eng.tensor_tensor(s_ap, kb_ap, q_ap, MULT)

        def expf(half, e_t=e_t):
            if EXP1:
                if half:
                    return
                s0, s1 = 0, EBLK
            else:
                s0, s1 = ((0, 4 * NPOS), (4 * NPOS, EBLK))[half]
            nc.scalar.activation(
                _fap(e_t, s0, [[1, s1 - s0]]),
                _fap(e_t, s0, [[1, s1 - s0]]).bitcast(F16),
                mybir.ActivationFunctionType.Exp)

        def mmul(half, e_t=e_t, m_t=m_t, mt=mt, g=g):
            if mt == 0:
                par = g % 2
                if SPLIT_MT1:
                    i0, ni = ((0, 4), (4, 3))[half]
                elif half:
                    return
                else:
                    i0, ni = 0, K
                e_ap = _fap(e_t, i0 * NPOS, [[NPOS, ni], [W, ROWS], [1, W]])
                v_ap = _fap(v1[0] if par else v0[0],
                            (g - par) + i0 * SW,
                            [[SW, ni], [SW, ROWS], [1, W]])
                m_ap = _fap(m_t, i0 * NPOS,
                            [[NPOS, ni], [W, ROWS], [1, W]])
                eng = (nc.gpsimd if g in POOL_M0 else nc.vector)
                eng.tensor_tensor(m_ap, e_ap, v_ap, MULT)
            else:
                par = g % 2
                if SPLIT_MT1:
                    i0, ni = ((0, 4), (4, 3))[half]
                elif half:
                    return
                else:
                    i0, ni = 0, K
                e_ap = _fap(e_t, i0 * NPOS, [[NPOS, ni], [W, ROWS], [1, W]])
                v_ap = _fap(v1[1] if par else v0[1],
                            g - par + i0 * SW,
                            [[SW, ni], [SW, ROWS], [1, W]])
                m_ap = _fap(m_t, i0 * NPOS,
                            [[NPOS, ni], [W, ROWS], [1, W]])
                eng = (nc.gpsimd if (g, half) in POOL_M1 else nc.vector)
                eng.tensor_tensor(m_ap, e_ap, v_ap, MULT)

        def accum(half, blk, e_t=e_t, m_t=m_t, mt=mt, last=last):
            # bind tiles at definition: the closure is queued in pend_num
            # and must keep THIS group's tiles, not the loop's latest.
            # chunks must stay within 512-col PSUM banks:
            # den at dn[0:576]   -> (0,512),(512,576)
            # num at dn[576:1152] -> (576,1024),(1024,1152)
            off = 0 if blk == 0 else NPOS
            chunks = ((0, 512), (512, NPOS)) if blk == 0 else \
                     ((0, 448), (448, NPOS))
            sls = (range(0, 4), range(4, K))[half]
            src_t = e_t if blk == 0 else m_t
            for sl in sls:
                for c0, c1 in chunks:
                    nc.tensor.matmul(
                        cx.dn[mt][:, off + c0:off + c1], cx.idb[:],
                        src_t[:, sl * NPOS + c0:sl * NPOS + c1],
                        start=False,
                        stop=(last and sl == K - 1 and half == 1),
                        skip_group_check=True)

        if "score" not in SKIP:
            score(0)
            score(1)
        if "exp" not in SKIP:
            expf(0)
            expf(1)
        if M_DELAY == 0:
            if "mmul" not in SKIP:
                mmul(0)
            if "accum" not in SKIP:
                accum(0, 0)
            if "mmul" not in SKIP:
                mmul(1)
            if "accum" not in SKIP:
                if NUM_DELAY == 0:
                    accum(0, EBLK)
                    accum(1, 0)
                    accum(1, EBLK)
                else:
                    accum(1, 0)
                    pend_num.append((accum, mt))
                    while len(pend_num) > NUM_DELAY:
                        fn, _ = pend_num.pop(0)
                        fn(0, EBLK)
                        fn(1, EBLK)
        else:
            if "accum" not in SKIP:
                accum(0, 0)
                accum(1, 0)
            pend_m.append((mmul, accum, mt))
            while len(pend_m) > M_DELAY:
                mf, af, _ = pend_m.pop(0)
                if "mmul" not in SKIP:
                    mf(0)
                if "accum" not in SKIP:
                    af(0, EBLK)
                if "mmul" not in SKIP:
                    mf(1)
                if "accum" not in SKIP:
                    af(1, EBLK)
        done.add((g, mt))
        if "out" in SKIP:
            continue
        if all((gg, mt) in done for gg in range(K)):
            while pend_m and pend_m[0][2] == mt:
                mf, af, _ = pend_m.pop(0)
                if "mmul" not in SKIP:
                    mf(0)
                if "accum" not in SKIP:
                    af(0, EBLK)
                if "mmul" not in SKIP:
                    mf(1)
                if "accum" not in SKIP:
                    af(1, EBLK)
            while pend_num and pend_num[0][1] == mt:
                fn, _ = pend_num.pop(0)
                fn(0, EBLK)
                fn(1, EBLK)
            rden = cx.ring.tile([128, NPOS], F32, tag="rden", bufs=2,
                                name=f"rden{mt}_{bn}")
            nc.vector.reciprocal(rden[:], cx.dn[mt][:, :NPOS])
            o_t = cx.ring.tile([128, NPOS], F32, tag="o", bufs=2,
                               name=f"o{mt}_{bn}")
            if DIV_VIA == "pool":
                # evict num to SBUF on ACT, divide on Pool (can't read PSUM)
                nev = cx.ring.tile([128, NPOS], F32, tag="nev", bufs=2,
                                   name=f"nev{mt}_{bn}")
                nc.scalar.copy(nev[:], cx.dn[mt][:, NPOS:])
                for c0, c1 in ((0, NPOS // 2), (NPOS // 2, NPOS)):
                    nc.gpsimd.tensor_tensor(o_t[:, c0:c1], nev[:, c0:c1],
                                            rden[:, c0:c1], MULT)
                    nc.sync.dma_start(out_d[mt * 128:(mt + 1) * 128, c0:c1],
                                      o_t[:, c0:c1])
            else:
                chunks = (((0, NPOS),) if DIV1
                          else ((0, NPOS // 2), (NPOS // 2, NPOS)))
                for c0, c1 in chunks:
                    nc.vector.tensor_tensor(o_t[:, c0:c1],
                                            cx.dn[mt][:, NPOS + c0:NPOS + c1],
                                            rden[:, c0:c1], MULT)
                    nc.sync.dma_start(out_d[mt * 128:(mt + 1) * 128, c0:c1],
                                      o_t[:, c0:c1])


def _build_graph(repeat=1):
    nc = bacc.Bacc("TRN2", target_bir_lowering=False, debug=False,
                   num_devices=N_CORES)

    GDT = {"f16": F16, "f32r": F32R, "f32": F32}[GEMM_DT]
    dram = (
        nc.declare_dram_parameter("x_slab", [CIN, NPAD], GDT, isOutput=False),
        nc.declare_dram_parameter("w_pack", [128, 12 * 128], GDT,
                                  isOutput=False),
        nc.declare_dram_parameter("bi_pack", [128, 2 * K + 128], F32,
                                  isOutput=False),
        nc.declare_dram_parameter("out", [COUT, NPOS], F32, isOutput=True),
    )

    with tile.TileContext(nc) as tc:
        with ExitStack() as stk:
            cx = _emit_prologue(nc, tc, stk)
            _emit_body(nc, tc, cx, dram, 0, warm=True)
            left = repeat - 1
            if left > 0:
                u = min(UNROLL, left)
                trips, rem = divmod(left, u)
                if trips > 0:
                    with tc.For_i(0, trips, 1, staggered_reset=STAGGER):
                        for j in range(u):
                            _emit_body(nc, tc, cx, dram, (j + 1) % 2)
                for j in range(rem):
                    _emit_body(nc, tc, cx, dram, (j + 1) % 2)

    nc.compile()
    return nc


def _prep_host(x, w_q, w_k, w_v, rel_h, rel_w):
    gnp = {"f16": np.float16, "f32r": np.float32,
           "f32": np.float32}[GEMM_DT]
    x = np.ascontiguousarray(x, np.float32)
    beta = np.zeros((COUT, K), np.float32)
    beta[:COUT // 2] = rel_h.reshape(COUT // 2, K)
    beta[COUT // 2:] = rel_w.reshape(COUT // 2, K)
    bi_pack = np.empty((128, 2 * K + 128), np.float32)
    for mt in range(2):
        bi_pack[:, mt * K:(mt + 1) * K] = beta[mt * 128:(mt + 1) * 128]
    bi_pack[:, 2 * K:] = np.eye(128, dtype=np.float32)
    # w layout: [kt][k|q|v][mt] blocks of 128 cols
    w_pack = np.empty((128, 12 * 128), gnp)
    ws = {"k": w_k, "q": w_q, "v": w_v}
    for kt in range(2):
        for wi, nm in enumerate("kqv"):
            for mt in range(2):
                blk = ((kt * 3) + wi) * 2 + mt
                w_pack[:, blk * 128:(blk + 1) * 128] = \
                    ws[nm].T[kt * 128:(kt + 1) * 128,
                             mt * 128:(mt + 1) * 128].astype(gnp)
    common = {"w_pack": w_pack, "bi_pack": bi_pack}
    in_maps = []
    for core in range(N_CORES):
        b, r0 = divmod(core, 4)
        r0 *= ROWS
        slab = np.zeros((CIN, SH, SW), np.float32)
        lo, hi = r0 - PAD, r0 + ROWS + PAD
        clo, chi = max(lo, 0), min(hi, H)
        slab[:, clo - lo:chi - lo, PAD:PAD + W] = x[b, :, clo:chi, :]
        in_maps.append({"x_slab": slab.reshape(CIN, NPAD).astype(gnp),
                        **common})
    return in_maps


def kernel(x, w_q, w_k, w_v, rel_h, rel_w):
    if "nc" not in _CACHED:
        _CACHED["nc"] = _build_graph()
    nc = _CACHED["nc"]
    in_maps = _prep_host(x, w_q, w_k, w_v, rel_h, rel_w)
    res = run_bass_kernel_spmd(nc, in_maps, core_ids=list(range(N_CORES)))
    _CACHED["exec_time_ns"] = res.exec_time_ns
    out = np.empty((B, COUT, H, W), np.float32)
    for core in range(N_CORES):
        b, r0 = divmod(core, 4)
        r0 *= ROWS
        out[b, :, r0:r0 + ROWS, :] = \
            res.results[core]["out"].reshape(COUT, ROWS, W)
    return out
